# revision 26
# baseline (speedup 1.0000x reference)
"""DigitCaps dynamic-routing kernel for 8 TRN2 NeuronCores.

Problem (hardcoded): x [256,1152,8] f32, W [1,1152,10,16,8] f32, 3 routing
iterations -> v [256,10,16,1] f32.

Strategy: shard the R=1152 routes 8-ways (144 per core), full batch B=256 on
every core.  u_hat is never materialized; each iteration streams W through
the TensorEngine with the softmax weights folded into x:
  s_c[o,b]  = sum_{(r,i)} WS_c[(r,i),o] * (en_c[r,b] * x[(r,i),b])     (PE)
  a_c[r,b]  = sum_i x[(r,i),b] * (sum_o WOTZ_c[(c,o),(r,i)] * v[co,b]) (PE+DVE+PE)
Routing logits live in TRANSPOSED layout [r, (c,b)] and v stays co-major, so
no PE transposes are ever needed.  en is replicated across the i axis with
selector matmuls (R8_t); the sum over i runs on the PE via indicator
matmuls (E8_t).  Capsules are processed in PAIRS so every matmul / DVE op
runs at the full 512-column PSUM-bank width (the paired s-matmul computes a
2x2 block product and only the diagonal blocks are kept).  All tensors are
fp16 except logits/s/PSUM (f32): fp16 measured ~1e-2 end-to-end (bf16 is
catastrophic at 5e-2); fp16 stationaries get FWL and fp16 DVE multiplies
run at 2x.  Iteration-0 (uniform routing) s is computed REDUNDANTLY on
every core from a replicated copy of x/W, so the first collective is the
AllReduce after iteration 1 - by then the collectives barrier has drained.
The final reduction is a ReduceScatter over capsule rows; each core returns
a [20, 256] shard the host reassembles.
"""

import sys

if "/opt/trn_rl_repo" not in sys.path:
    sys.path.insert(0, "/opt/trn_rl_repo")

import numpy as np

import concourse.bass as bass
import concourse.tile as tile
from concourse import bacc, mybir
from concourse.bass_utils import run_bass_kernel_spmd

F32 = mybir.dt.float32
F16 = mybir.dt.float16

NCORES = 8
B, R, C, O, I = 256, 1152, 10, 16, 8
RL = R // NCORES          # 144 routes per core
RI = RL * I               # 1152 (r,i) rows per core
NT = RI // 128            # 9 K-chunks of 128
NTF = R * I // 128        # 72 K-chunks for the full (replicated) x
CO = C * O                # 160
COS = CO // NCORES        # 20 capsule-rows in the final output shard

AP = bass.AP
Exp = mybir.ActivationFunctionType.Exp
AX = mybir.AxisListType.X
ADD = mybir.AluOpType.add
MAX = mybir.AluOpType.max


def _insert_bcast(base, pos, count):
    """Insert a step-0 (broadcast) free dim into an existing AP at index pos."""
    dims = list(base.ap)
    dims.insert(pos, [0, count])
    return AP(tensor=base.tensor, offset=base.offset, ap=dims)


def build_kernel(n_iters: int, collectives: bool = True):
    assert n_iters >= 2
    nc = bacc.Bacc("TRN2", target_bir_lowering=False, debug=False,
                   num_devices=NCORES)

    xt_in = nc.dram_tensor("xt", [128, NT, 2 * B], F16,
                           kind="ExternalInput")
    ws_in = nc.dram_tensor("ws", [128, NT, CO], F16, kind="ExternalInput")
    xtf_in = nc.dram_tensor("xtf", [128, NTF, B], F16, kind="ExternalInput")
    wsf_in = nc.dram_tensor("wsf", [128, NTF, CO], F16, kind="ExternalInput")
    wz_in = nc.dram_tensor("wz", [128, C, RI], F16, kind="ExternalInput")
    wz2_in = nc.dram_tensor("wz2", [32, 2, RI], F16, kind="ExternalInput")
    sel_in = nc.dram_tensor("sel", [128, 2, NT, 128], F16,
                            kind="ExternalInput")
    out = nc.dram_tensor("out", [COS, B], F32, kind="ExternalOutput")

    with tile.TileContext(nc) as tc:
        with (
            tc.tile_pool(name="stat", bufs=1) as stat,
            tc.tile_pool(name="work", bufs=1) as work,
            tc.tile_pool(name="drp", bufs=2) as drp,
            tc.tile_pool(name="s0xp", bufs=1) as s0xp,
            tc.tile_pool(name="s0wp", bufs=1) as s0wp,
            tc.tile_pool(name="cast", bufs=6) as castp,
            tc.tile_pool(name="yp", bufs=6) as yp,
            tc.tile_pool(name="prp", bufs=6) as prp,
            tc.tile_pool(name="dram", bufs=2, space="DRAM") as dram,
            tc.tile_pool(name="ps_big", bufs=3, space="PSUM") as ps_big,
            tc.tile_pool(name="ps_sp", bufs=2, space="PSUM") as ps_sp,
            tc.tile_pool(name="ps_a", bufs=2, space="PSUM") as ps_a,
            tc.tile_pool(name="ps_a2", bufs=1, space="PSUM") as ps_a2,
        ):
            # ---- static SBUF tensors ----
            XT = stat.tile([128, NT, 2 * B], F16)   # x^T duplicated pair-wide
            WS = stat.tile([128, NT, CO], F16)      # W as lhsT for s-matmul
            WZ = stat.tile([128, C, RI], F16)       # zero-padded W^T, c<8
            WZ2 = stat.tile([32, 2, RI], F16)       # same for c=8,9
            SEL = stat.tile([128, 2, NT, 128], F16)  # R8_t / E8_t selectors
            def late_loads():
                # everything the iteration phases need; issued AFTER the
                # s0 stream so the first s0 blocks reach SBUF immediately
                nc.sync.dma_start(out=XT, in_=xt_in[:])
                nc.sync.dma_start(out=SEL, in_=sel_in[:])
                nc.gpsimd.dma_start(out=WS, in_=ws_in[:])
                nc.gpsimd.dma_start(out=WZ2, in_=wz2_in[:])
                nc.scalar.dma_start(out=WZ[:, 0:5, :], in_=wz_in[:, 0:5, :])
                nc.gpsimd.dma_start(out=WZ[:, 5:10, :],
                                    in_=wz_in[:, 5:10, :])

            # routing logits, transposed layout: [r%128, c, b] + 16-row tail
            BL1 = stat.tile([128, C, B], F32)
            BL2 = stat.tile([16, C, B], F32)
            E1 = stat.tile([128, C, B], F16)
            E2 = stat.tile([16, C, B], F16)
            EF1 = stat.tile([128, C, B], F32)
            EF2 = stat.tile([16, C, B], F32)
            MX1 = stat.tile([128, B], F32)
            MX2 = stat.tile([16, B], F32)
            Z1 = stat.tile([128, B], F32)
            Z2 = stat.tile([16, B], F32)
            ZH1 = stat.tile([128, B], F16)
            ZH2 = stat.tile([16, B], F16)
            EN1 = stat.tile([128, C, B], F16)
            EN2 = stat.tile([16, C, B], F16)
            # v (squashed capsule outputs), co-major fp16
            VC1 = stat.tile([128, B], F16)
            VC2 = stat.tile([32, B], F16)
            NEG40 = stat.tile([128, 1], F32)
            nc.vector.memset(NEG40, -40.0)

            def _squash(sf, vout, scale, shp, tag):
                sq = work.tile(shp, F32, tag=f"sq_sq{tag}")
                ab = work.tile(shp, F32, tag=f"sq_ab{tag}")
                den = work.tile(shp, F32, tag=f"sq_den{tag}")
                if scale != 1.0:
                    nc.scalar.mul(sf, sf, scale)
                nc.scalar.square(sq[:, :], sf)
                nc.scalar.sqrt(ab[:, :], sq[:, :])
                nc.vector.tensor_scalar_add(den[:, :], sq[:, :], 1.0)
                nc.vector.reciprocal_approx_fast(den[:, :], den[:, :])
                nc.vector.tensor_mul(ab[:, :], ab[:, :], den[:, :])
                nc.vector.tensor_mul(vout[:, :], ab[:, :], sf)

            def s0_local():
                """v0 = squash(0.1 * sum_{ALL r} u_hat), computed redundantly
                on every core from the replicated full x/W (no collective)."""
                p1t = ps_big.tile([128, 512], F32, tag="big")
                p2t = ps_big.tile([128, 512], F32, tag="big")
                p1 = p1t[:, 0:256]
                p2 = p2t[0:32, 0:256]
                NB = 8                       # chunks per streamed block
                nblk = NTF // NB
                for blk in range(nblk):
                    xb = s0xp.tile([128, NB, B], F16, tag=f"s0x{blk % 4}")
                    wb = s0wp.tile([128, NB, CO], F16, tag=f"s0w{blk % 4}")
                    t0 = blk * NB
                    h = NB // 2
                    nc.sync.dma_start(out=xb[:, 0:h, :],
                                      in_=xtf_in[:, t0:t0 + h, :])
                    nc.gpsimd.dma_start(out=xb[:, h:NB, :],
                                        in_=xtf_in[:, t0 + h:t0 + NB, :])
                    nc.scalar.dma_start(out=wb,
                                        in_=wsf_in[:, t0:t0 + NB, :])
                    if blk == 2:
                        late_loads()
                    for j in range(NB):
                        t = t0 + j
                        nc.tensor.matmul(p1, wb[:, j, 0:128], xb[:, j, :],
                                         start=(t == 0),
                                         stop=(t == NTF - 1))
                        nc.tensor.matmul(p2, wb[:, j, 128:160], xb[:, j, :],
                                         start=(t == 0),
                                         stop=(t == NTF - 1))
                s1 = work.tile([128, B], F32, tag="s0_1")
                s2 = work.tile([32, B], F32, tag="s0_2")
                nc.scalar.copy(s1[:, :], p1)
                nc.scalar.copy(s2[:, :], p2)
                _squash(s1[:, :], VC1, 0.1, [128, B], "a")
                _squash(s2[:, :], VC2, 0.1, [32, B], "b")

            def reduce_s(writes, last):
                """writes: list of (psum AP [16,B], co0). AllReduce or RS.

                Bounce buffer is co-major [CO, B]; every DMA is contiguous.
                """
                b_in = dram.tile([CO, B], F32, tag="arin")
                for pap, rows, co0 in writes:
                    sb = drp.tile([32, B], F32, tag=f"sdr{co0 % 64}")
                    nc.scalar.copy(sb[0:rows, :], pap)
                    nc.sync.dma_start(out=b_in[co0:co0 + 16, :],
                                      in_=sb[rows - 16:rows, :])
                if last and collectives:
                    b_out = dram.tile([COS, B], F32, tag="rsout")
                    nc.gpsimd.collective_compute(
                        "ReduceScatter", ADD,
                        replica_groups=[list(range(NCORES))],
                        ins=[b_in[:].opt()], outs=[b_out[:].opt()])
                elif collectives:
                    b_out = dram.tile([CO, B], F32, tag="arout")
                    nc.gpsimd.collective_compute(
                        "AllReduce", ADD,
                        replica_groups=[list(range(NCORES))],
                        ins=[b_in[:].opt()], outs=[b_out[:].opt()])
                else:
                    b_out = dram.tile([COS, B] if last else [CO, B], F32,
                                      tag="arout")
                    src = b_in[0:COS, :] if last else b_in[:]
                    nc.sync.dma_start(out=b_out[:], in_=src)
                return b_out

            def squash_full(b_out):
                """load s (co-major) from bounce, squash into VC1/VC2."""
                s1 = work.tile([128, B], F32, tag="sq_s1")
                s2 = work.tile([32, B], F32, tag="sq_s2")
                nc.sync.dma_start(out=s1, in_=b_out[0:128, :])
                nc.sync.dma_start(out=s2, in_=b_out[128:160, :])
                _squash(s1[:, :], VC1, 1.0, [128, B], "a")
                _squash(s2[:, :], VC2, 1.0, [32, B], "b")

            def squash_shard(b_out):
                """final: squash this core's [20, B] shard and emit."""
                s = work.tile([COS, B], F32, tag="fs_s")
                vv = work.tile([COS, B], F32, tag="fs_v")
                nc.sync.dma_start(out=s, in_=b_out[:])
                _squash(s[:, :], vv, 1.0, [COS, B], "f")
                nc.sync.dma_start(out=out[:], in_=vv[:, :])

            def fold(dst, src_psum, xv, use_cast):
                """dst[f16] = src_psum[f32] * xv, optionally via a scalar
                fp16 cast so the DVE multiply runs in 2x mode."""
                if use_cast:
                    mh = castp.tile([128, 512], F16, tag="mh")
                    nc.scalar.copy(mh[:, 0:512], src_psum)
                    nc.vector.tensor_mul(dst, mh[:, 0:512], xv)
                else:
                    nc.vector.tensor_mul(dst, src_psum, xv)

            def a_phase(first):
                """logits (+)= a;  a_c[r,b] = sum_i x*(W_c^T v_c).

                Capsule pairs share each 512-wide psum bank; chunk t=8 (the
                16-row logit tail) runs first so its softmax half can overlap
                the rest of the phase.
                """
                for cp in range(5):
                    c0 = 2 * cp
                    pa1 = ps_a.tile([128, 512], F32, tag="pa")
                    pa2 = ps_a2.tile([16, 512], F32, tag="pa2")
                    for t in (8, 0, 1, 2, 3, 4, 5, 6, 7):
                        mp = ps_big.tile([128, 512], F32, tag="big")
                        for j in (0, 1):
                            if cp < 4:
                                nc.tensor.matmul(
                                    mp[:, j * 256:(j + 1) * 256],
                                    WZ[:, c0 + j, 128 * t:128 * (t + 1)],
                                    VC1[:, :], start=True, stop=True)
                            else:
                                nc.tensor.matmul(
                                    mp[:, j * 256:(j + 1) * 256],
                                    WZ2[:, j, 128 * t:128 * (t + 1)],
                                    VC2[:, :], start=True, stop=True)
                        xv = XT[:, t, :]
                        pr = prp.tile([128, 512], F16, tag="prod")
                        fold(pr[:, 0:512], mp[:, 0:512], xv, cp < 3)
                        if t < 8:
                            nc.tensor.matmul(pa1[:, :], SEL[:, 1, t, :],
                                             pr[:, 0:512],
                                             start=(t == 0), stop=(t == 7))
                        else:
                            nc.tensor.matmul(pa2[:, :], SEL[:, 1, 8, 0:16],
                                             pr[:, 0:512],
                                             start=True, stop=True)
                            # tail logits done early -> softmax half overlaps
                            blv = BL2[:, c0:c0 + 2, :].rearrange(
                                "p c b -> p (c b)")
                            if first:
                                nc.scalar.copy(blv, pa2[:, :])
                            else:
                                nc.vector.tensor_add(blv, blv, pa2[:, :])
                    blv = BL1[:, c0:c0 + 2, :].rearrange("p c b -> p (c b)")
                    if first:
                        nc.scalar.copy(blv, pa1[:, :])
                    else:
                        nc.vector.tensor_add(blv, blv, pa1[:, :])

            def softmax(first):
                """en = softmax_c(logits) -> fp16, transposed layout.

                first=True: logits are bounded (|a1|<~90, row-max>-12), so
                skip the max pass and use exp(b-40) in f32.
                """
                def red(dst, src3, op):
                    v = src3.rearrange("p c b -> p b c")
                    nc.vector.tensor_reduce(dst[:, :], v[:, :, :],
                                            axis=AX, op=op)

                def zsum(dst, src3):
                    # sum over c on GpSimd (frees the DVE): tree of adds
                    nc.gpsimd.tensor_add(dst[:, :], src3[:, 0, :],
                                         src3[:, 1, :])
                    for c in range(2, C):
                        nc.gpsimd.tensor_add(dst[:, :], dst[:, :],
                                             src3[:, c, :])

                if first:
                    nc.scalar.activation(EF2[:, :, :], BL2[:, :, :], Exp,
                                         bias=NEG40[0:16, 0:1])
                    red(Z2, EF2[:, :, :], ADD)
                    nc.vector.reciprocal_approx_fast(Z2[:, :], Z2[:, :])
                    nc.vector.tensor_mul(EN2[:, :, :], EF2[:, :, :],
                                         _insert_bcast(Z2[:, :], 1, C))
                    nc.scalar.activation(EF1[:, :, :], BL1[:, :, :], Exp,
                                         bias=NEG40[:, 0:1])
                    red(Z1, EF1[:, :, :], ADD)
                    nc.vector.reciprocal_approx_fast(Z1[:, :], Z1[:, :])
                    nc.vector.tensor_mul(EN1[:, :, :], EF1[:, :, :],
                                         _insert_bcast(Z1[:, :], 1, C))
                    return
                red(MX2, BL2[:, :, :], MAX)
                nc.vector.tensor_sub(E2[:, :, :], BL2[:, :, :],
                                     _insert_bcast(MX2[:, :], 1, C))
                nc.scalar.activation(E2[:, :, :], E2[:, :, :], Exp)
                red(Z2, E2[:, :, :], ADD)
                nc.vector.reciprocal_approx_fast(Z2[:, :], Z2[:, :])
                nc.scalar.copy(ZH2[:, :], Z2[:, :])
                nc.vector.tensor_mul(EN2[:, :, :], E2[:, :, :],
                                     _insert_bcast(ZH2[:, :], 1, C))
                red(MX1, BL1[:, :, :], MAX)
                nc.vector.tensor_sub(E1[:, :, :], BL1[:, :, :],
                                     _insert_bcast(MX1[:, :], 1, C))
                nc.scalar.activation(E1[:, :, :], E1[:, :, :], Exp)
                red(Z1, E1[:, :, :], ADD)
                nc.vector.reciprocal_approx_fast(Z1[:, :], Z1[:, :])
                nc.scalar.copy(ZH1[:, :], Z1[:, :])
                nc.vector.tensor_mul(EN1[:, :, :], E1[:, :, :],
                                     _insert_bcast(ZH1[:, :], 1, C))

            def s_phase():
                """y = repl(en)*x; paired block s-matmul [32,512] per (cp,t);
                only the diagonal 16x256 blocks of each pair are kept."""
                writes = []
                for cp in range(5):
                    c0 = 2 * cp
                    spp = ps_sp.tile([32, 512], F32, tag="sp")
                    for t in range(NT):
                        rp = ps_big.tile([128, 512], F32, tag="big")
                        if t < 8:
                            nc.tensor.matmul(
                                rp[:, 0:512], SEL[:, 0, t, :],
                                EN1[:, c0:c0 + 2, :].rearrange(
                                    "p c b -> p (c b)"),
                                start=True, stop=True)
                        else:
                            nc.tensor.matmul(
                                rp[:, 0:512], SEL[0:16, 0, 8, :],
                                EN2[:, c0:c0 + 2, :].rearrange(
                                    "p c b -> p (c b)"),
                                start=True, stop=True)
                        xv = XT[:, t, :]
                        y = yp.tile([128, 512], F16, tag="y")
                        fold(y[:, 0:512], rp[:, 0:512], xv, cp < 3)
                        nc.tensor.matmul(
                            spp[:, :], WS[:, t, 32 * cp:32 * (cp + 1)],
                            y[:, 0:512], start=(t == 0), stop=(t == NT - 1))
                    writes.append((spp[0:16, 0:256], 16, 32 * cp))
                    writes.append((spp[0:32, 256:512], 32, 32 * cp + 16))
                return writes

            def pe_warm(n):
                """keep the PE clock-gate warm across engine-idle windows"""
                jt = ps_big.tile([128, 512], F32, tag="big")
                for _ in range(n):
                    nc.tensor.matmul(jt[:, 0:256], SEL[:, 0, 0, :],
                                     XT[:, 0, 0:256], start=True, stop=True)

            def pe_warm(n):
                """keep the PE clock-gate warm across engine-idle windows;
                sized to fit inside the window so real matmuls never queue
                behind the filler"""
                jt = ps_big.tile([128, 512], F32, tag="big")
                for _ in range(n):
                    nc.tensor.matmul(jt[:, 0:256], SEL[:, 0, 0, :],
                                     XT[:, 0, 0:256], start=True, stop=True)

            # ---------------- routing ----------------
            if collectives:
                # tiny collective issued first: absorbs the one-time CC
                # barrier + stream-start latency off the critical path
                d_in = dram.tile([8, 8], F32, tag="warm_in")
                d_out = dram.tile([8, 8], F32, tag="warm_out")
                wtile = work.tile([8, 8], F32, tag="warm")
                nc.vector.memset(wtile, 0.0)
                nc.scalar.dma_start(out=d_in[:], in_=wtile[:, :])
                nc.gpsimd.collective_compute(
                    "AllReduce", ADD,
                    replica_groups=[list(range(NCORES))],
                    ins=[d_in[:].opt()], outs=[d_out[:].opt()])
            s0_local()
            for it in range(1, n_iters):
                a_phase(first=(it == 1))
                pe_warm(40)
                softmax(first=(it == 1))
                writes = s_phase()
                last = (it == n_iters - 1)
                bout = reduce_s(writes, last=last)
                if not last:
                    pe_warm(70)
                    squash_full(bout)
                else:
                    squash_shard(bout)

    nc.compile()
    return nc


def prep_inputs(x: np.ndarray, W: np.ndarray):
    """Host-side layout prep. Returns per-core input dicts."""
    W = W[0]  # [R, C, O, I]
    f16 = np.float16
    # selector matrices, shared by all cores
    p = np.arange(128)[:, None]
    m = np.arange(128)[None, :]
    sel = np.zeros((128, 2, NT, 128), np.float32)
    for t in range(NT):
        sel[:, 0, t, :] = (p == (16 * t + m // 8) % 128) & \
            ((16 * t + m // 8) // 128 == t // 8)
        sel[:, 1, t, :] = (m == (16 * t + p // 8) % 128) & \
            ((16 * t + p // 8) // 128 == t // 8)
    selh = np.ascontiguousarray(sel.astype(f16))
    # full (replicated) x / W for the local iteration-0 pass
    xtf = np.transpose(x, (1, 2, 0)).reshape(NTF, 128, B)
    xtf = np.ascontiguousarray(np.transpose(xtf, (1, 0, 2))).astype(f16)
    wsf = np.transpose(W.reshape(NTF, 16, C, O, I), (0, 1, 4, 2, 3))
    wsf = wsf.reshape(NTF, 128, CO)
    wsf = np.ascontiguousarray(np.transpose(wsf, (1, 0, 2))).astype(f16)
    in_maps = []
    for k in range(NCORES):
        rs = slice(k * RL, (k + 1) * RL)
        xk = np.ascontiguousarray(x[:, rs, :])      # [B, RL, I]
        wk = np.ascontiguousarray(W[rs])            # [RL, C, O, I]
        xt = np.transpose(xk, (1, 2, 0)).reshape(NT, 128, B)
        xt = np.transpose(xt, (1, 0, 2))            # [128, NT, B]
        xt = np.concatenate([xt, xt], axis=2)       # [128, NT, 2B] dup
        # ws[p, t, c*16+o] = W[16t + p//8, c, o, p%8]
        wsk = np.transpose(wk.reshape(NT, 16, C, O, I), (0, 1, 4, 2, 3))
        wsk = wsk.reshape(NT, 128, CO)
        wsk = np.transpose(wsk, (1, 0, 2))          # [128, NT, CO]
        # wot[o, c, r*8+i] = W[r, c, o, i]; zero-pad into (c8, o) rows
        wot = np.transpose(wk, (2, 1, 0, 3)).reshape(O, C, RI)
        wz = np.zeros((128, C, RI), np.float32)
        for c in range(8):
            wz[16 * c:16 * (c + 1), c, :] = wot[:, c, :]
        wz2 = np.zeros((32, 2, RI), np.float32)
        for c in (8, 9):
            wz2[16 * (c - 8):16 * (c - 7), c - 8, :] = wot[:, c, :]
        in_maps.append({
            "xt": np.ascontiguousarray(xt).astype(f16),
            "ws": np.ascontiguousarray(wsk).astype(f16),
            "xtf": xtf,
            "wsf": wsf,
            "wz": np.ascontiguousarray(wz).astype(f16),
            "wz2": np.ascontiguousarray(wz2).astype(f16),
            "sel": selh,
        })
    return in_maps


_CACHE = {}


def _get_nc(n_iters: int):
    if n_iters not in _CACHE:
        _CACHE[n_iters] = build_kernel(n_iters)
    return _CACHE[n_iters]


def kernel(x, W, num_iterations, _trace=False):
    n = int(num_iterations)
    assert n >= 2, "n_iters==1 not built (problem uses 3)"
    nc = _get_nc(n)
    in_maps = prep_inputs(np.asarray(x, dtype=np.float32),
                          np.asarray(W, dtype=np.float32))
    res = run_bass_kernel_spmd(nc, in_maps, list(range(NCORES)),
                               trace=_trace)
    full = np.concatenate([res.results[k]["out"] for k in range(NCORES)],
                          axis=0)                       # [160, B] co-major
    v = np.transpose(full.reshape(C, O, B), (2, 0, 1))[..., None]
    kernel.last_results = res
    return v.astype(np.float32)


# revision 28
# speedup vs baseline: 1.2764x; 1.2764x over previous
"""DigitCaps dynamic-routing kernel for 8 TRN2 NeuronCores.

Problem (hardcoded): x [256,1152,8] f32, W [1,1152,10,16,8] f32, 3 routing
iterations -> v [256,10,16,1] f32.

Strategy: shard the R=1152 routes 8-ways (144 per core), full batch B=256 on
every core.  u_hat is never materialized; each iteration streams W through
the TensorEngine with the softmax weights folded into x:
  s_c[o,b]  = sum_{(r,i)} WS_c[(r,i),o] * (en_c[r,b] * x[(r,i),b])     (PE)
  a_c[r,b]  = sum_i x[(r,i),b] * (sum_o WOTZ_c[(c,o),(r,i)] * v[co,b]) (PE+DVE+PE)
Routing logits live in TRANSPOSED layout [r, (c,b)] and v stays co-major, so
no PE transposes are ever needed.  en is replicated across the i axis with
selector matmuls (R8_t); the sum over i runs on the PE via indicator
matmuls (E8_t).  Capsules are processed in PAIRS so every matmul / DVE op
runs at the full 512-column PSUM-bank width (the paired s-matmul computes a
2x2 block product and only the diagonal blocks are kept).  All tensors are
fp16 except logits/s/PSUM (f32): fp16 measured ~1e-2 end-to-end (bf16 is
catastrophic at 5e-2); fp16 stationaries get FWL and fp16 DVE multiplies
run at 2x.  Iteration-0 (uniform routing) s is computed REDUNDANTLY on
every core from a replicated copy of x/W, so the first collective is the
AllReduce after iteration 1 - by then the collectives barrier has drained.
The final reduction is a ReduceScatter over capsule rows; each core returns
a [20, 256] shard the host reassembles.
"""

import sys

if "/opt/trn_rl_repo" not in sys.path:
    sys.path.insert(0, "/opt/trn_rl_repo")

import numpy as np

import concourse.bass as bass
import concourse.tile as tile
from concourse import bacc, mybir
from concourse.bass_utils import run_bass_kernel_spmd

F32 = mybir.dt.float32
F16 = mybir.dt.float16

NCORES = 8
B, R, C, O, I = 256, 1152, 10, 16, 8
RL = R // NCORES          # 144 routes per core
RI = RL * I               # 1152 (r,i) rows per core
NT = RI // 128            # 9 K-chunks of 128
NTF = R * I // 128        # 72 K-chunks for the full (replicated) x
CO = C * O                # 160
COS = CO // NCORES        # 20 capsule-rows in the final output shard

AP = bass.AP
Exp = mybir.ActivationFunctionType.Exp
AX = mybir.AxisListType.X
ADD = mybir.AluOpType.add
MAX = mybir.AluOpType.max


def _insert_bcast(base, pos, count):
    """Insert a step-0 (broadcast) free dim into an existing AP at index pos."""
    dims = list(base.ap)
    dims.insert(pos, [0, count])
    return AP(tensor=base.tensor, offset=base.offset, ap=dims)


def build_kernel(n_iters: int, collectives: bool = True):
    assert n_iters >= 2
    nc = bacc.Bacc("TRN2", target_bir_lowering=False, debug=False,
                   num_devices=NCORES)

    xtf_in = nc.dram_tensor("xtf", [128, NTF, B], F16, kind="ExternalInput")
    wsf_in = nc.dram_tensor("wsf", [128, NTF, CO], F16, kind="ExternalInput")
    wz_in = nc.dram_tensor("wz", [128, 5, RI], F16, kind="ExternalInput")
    wz2_in = nc.dram_tensor("wz2", [32, RI], F16, kind="ExternalInput")
    mk_in = nc.dram_tensor("mk", [128, 512], F16, kind="ExternalInput")
    sel_in = nc.dram_tensor("sel", [128, 2, NT, 128], F16,
                            kind="ExternalInput")
    out = nc.dram_tensor("out", [COS, B], F32, kind="ExternalOutput")

    with tile.TileContext(nc) as tc:
        with (
            tc.tile_pool(name="stat", bufs=1) as stat,
            tc.tile_pool(name="work", bufs=1) as work,
            tc.tile_pool(name="drp", bufs=2) as drp,
            tc.tile_pool(name="s0xp", bufs=1) as s0xp,
            tc.tile_pool(name="s0wp", bufs=1) as s0wp,
            tc.tile_pool(name="cast", bufs=6) as castp,
            tc.tile_pool(name="yp", bufs=6) as yp,
            tc.tile_pool(name="prp", bufs=6) as prp,
            tc.tile_pool(name="dram", bufs=2, space="DRAM") as dram,
            tc.tile_pool(name="ps_big", bufs=3, space="PSUM") as ps_big,
            tc.tile_pool(name="ps_sp", bufs=2, space="PSUM") as ps_sp,
            tc.tile_pool(name="ps_a", bufs=2, space="PSUM") as ps_a,
            tc.tile_pool(name="ps_a2", bufs=1, space="PSUM") as ps_a2,
        ):
            # ---- static SBUF tensors ----
            XT = stat.tile([128, NT, B], F16)       # own-shard x^T (block 0)
            WS = stat.tile([128, NT, CO], F16)      # own-shard W (block 0)
            WZ = stat.tile([128, 5, RI], F16)       # pair-summed W^T, cp<4
            WZ2 = stat.tile([32, RI], F16)          # same for cp=4 (c=8,9)
            SEL = stat.tile([128, 2, NT, 128], F16)  # R8_t / E8_t selectors
            VCP = stat.tile([128, 512], F16)        # masked pair-dup v
            VCP2 = stat.tile([32, 512], F16)
            MK = stat.tile([128, 512], F16)         # pair block mask
            def late_loads():
                # everything the iteration phases need; issued AFTER the
                # s0 stream so the first s0 blocks reach SBUF immediately
                nc.sync.dma_start(out=SEL, in_=sel_in[:])
                nc.sync.dma_start(out=MK, in_=mk_in[:])
                nc.gpsimd.dma_start(out=WZ2, in_=wz2_in[:])
                nc.scalar.dma_start(out=WZ[:, 0:3, :], in_=wz_in[:, 0:3, :])
                nc.gpsimd.dma_start(out=WZ[:, 3:5, :], in_=wz_in[:, 3:5, :])

            # routing logits, transposed layout: [r%128, c, b] + 16-row tail
            BL1 = stat.tile([128, C, B], F32)
            BL2 = stat.tile([16, C, B], F32)
            E1 = stat.tile([128, C, B], F16)
            E2 = stat.tile([16, C, B], F16)
            EF1 = stat.tile([128, C, B], F32)
            EF2 = stat.tile([16, C, B], F32)
            MX1 = stat.tile([128, B], F32)
            MX2 = stat.tile([16, B], F32)
            Z1 = stat.tile([128, B], F32)
            Z2 = stat.tile([16, B], F32)
            ZH1 = stat.tile([128, B], F16)
            ZH2 = stat.tile([16, B], F16)
            EN1 = stat.tile([128, C, B], F16)
            EN2 = stat.tile([16, C, B], F16)
            # v (squashed capsule outputs), co-major fp16
            VC1 = stat.tile([128, B], F16)
            VC2 = stat.tile([32, B], F16)
            NEG40 = stat.tile([128, 1], F32)
            nc.vector.memset(NEG40, -40.0)

            def _squash(sf, vout, scale, shp, tag):
                sq = work.tile(shp, F32, tag=f"sq_sq{tag}")
                ab = work.tile(shp, F32, tag=f"sq_ab{tag}")
                den = work.tile(shp, F32, tag=f"sq_den{tag}")
                if scale != 1.0:
                    nc.scalar.mul(sf, sf, scale)
                nc.scalar.square(sq[:, :], sf)
                nc.scalar.sqrt(ab[:, :], sq[:, :])
                nc.vector.tensor_scalar_add(den[:, :], sq[:, :], 1.0)
                nc.vector.reciprocal_approx_fast(den[:, :], den[:, :])
                nc.vector.tensor_mul(ab[:, :], ab[:, :], den[:, :])
                nc.vector.tensor_mul(vout[:, :], ab[:, :], sf)

            def build_vcp():
                """v duplicated across the pair halves then masked so each
                16-row o-band is live only in its own 256-column half; one
                tensor serves every capsule pair (mask is p%32-periodic)."""
                nc.vector.tensor_mul(VCP[:, :],
                                     _insert_bcast(VC1[:, :], 1, 2),
                                     MK[:, :])
                nc.vector.tensor_mul(VCP2[:, :],
                                     _insert_bcast(VC2[:, :], 1, 2),
                                     MK[0:32, :])

            def s0_local():
                """v0 = squash(0.1 * sum_{ALL r} u_hat), computed redundantly
                on every core from the replicated full x/W (no collective)."""
                p1t = ps_big.tile([128, 512], F32, tag="big")
                p2t = ps_big.tile([128, 512], F32, tag="big")
                p1 = p1t[:, 0:256]
                p2 = p2t[0:32, 0:256]
                NB = 9                       # chunks per streamed block
                nblk = NTF // NB
                for blk in range(nblk):
                    if blk == 0:
                        # block 0 is this core's own R-shard (host rolls the
                        # replicated x/W so the shard comes first): land it
                        # in the persistent XT/WS used by every iteration
                        xb, wb = XT, WS
                    else:
                        xb = s0xp.tile([128, NB, B], F16,
                                       tag=f"s0x{blk % 4}")
                        wb = s0wp.tile([128, NB, CO], F16,
                                       tag=f"s0w{blk % 4}")
                    t0 = blk * NB
                    h = NB // 2
                    nc.sync.dma_start(out=xb[:, 0:h, :],
                                      in_=xtf_in[:, t0:t0 + h, :])
                    nc.gpsimd.dma_start(out=xb[:, h:NB, :],
                                        in_=xtf_in[:, t0 + h:t0 + NB, :])
                    nc.scalar.dma_start(out=wb,
                                        in_=wsf_in[:, t0:t0 + NB, :])
                    if blk == 2:
                        late_loads()
                    for j in range(NB):
                        t = t0 + j
                        nc.tensor.matmul(p1, wb[:, j, 0:128], xb[:, j, :],
                                         start=(t == 0),
                                         stop=(t == NTF - 1))
                        nc.tensor.matmul(p2, wb[:, j, 128:160], xb[:, j, :],
                                         start=(t == 0),
                                         stop=(t == NTF - 1))
                s1 = work.tile([128, B], F32, tag="s0_1")
                s2 = work.tile([32, B], F32, tag="s0_2")
                nc.scalar.copy(s1[:, :], p1)
                nc.scalar.copy(s2[:, :], p2)
                _squash(s1[:, :], VC1, 0.1, [128, B], "a")
                _squash(s2[:, :], VC2, 0.1, [32, B], "b")
                build_vcp()

            def reduce_s(writes, last):
                """writes: list of (psum AP [16,B], co0). AllReduce or RS.

                Bounce buffer is co-major [CO, B]; every DMA is contiguous.
                """
                b_in = dram.tile([CO, B], F32, tag="arin")
                for pap, rows, co0 in writes:
                    sb = drp.tile([32, B], F32, tag=f"sdr{co0 % 64}")
                    nc.scalar.copy(sb[0:rows, :], pap)
                    nc.sync.dma_start(out=b_in[co0:co0 + 16, :],
                                      in_=sb[rows - 16:rows, :])
                if last and collectives:
                    b_out = dram.tile([COS, B], F32, tag="rsout")
                    nc.gpsimd.collective_compute(
                        "ReduceScatter", ADD,
                        replica_groups=[list(range(NCORES))],
                        ins=[b_in[:].opt()], outs=[b_out[:].opt()])
                elif collectives:
                    b_out = dram.tile([CO, B], F32, tag="arout")
                    nc.gpsimd.collective_compute(
                        "AllReduce", ADD,
                        replica_groups=[list(range(NCORES))],
                        ins=[b_in[:].opt()], outs=[b_out[:].opt()])
                else:
                    b_out = dram.tile([COS, B] if last else [CO, B], F32,
                                      tag="arout")
                    src = b_in[0:COS, :] if last else b_in[:]
                    nc.sync.dma_start(out=b_out[:], in_=src)
                return b_out

            def squash_full(b_out):
                """load s (co-major) from bounce, squash into VC1/VC2."""
                s1 = work.tile([128, B], F32, tag="sq_s1")
                s2 = work.tile([32, B], F32, tag="sq_s2")
                nc.sync.dma_start(out=s1, in_=b_out[0:128, :])
                nc.sync.dma_start(out=s2, in_=b_out[128:160, :])
                _squash(s1[:, :], VC1, 1.0, [128, B], "a")
                _squash(s2[:, :], VC2, 1.0, [32, B], "b")
                build_vcp()

            def squash_shard(b_out):
                """final: squash this core's [20, B] shard and emit."""
                s = work.tile([COS, B], F32, tag="fs_s")
                vv = work.tile([COS, B], F32, tag="fs_v")
                nc.sync.dma_start(out=s, in_=b_out[:])
                _squash(s[:, :], vv, 1.0, [COS, B], "f")
                nc.sync.dma_start(out=out[:], in_=vv[:, :])

            def fold(dst, src_psum, xv, use_cast):
                """dst[f16] = src_psum[f32] * xv, optionally via a scalar
                fp16 cast so the DVE multiply runs in 2x mode."""
                if use_cast:
                    mh = castp.tile([128, 512], F16, tag="mh")
                    nc.scalar.copy(mh[:, 0:512], src_psum)
                    nc.vector.tensor_mul(dst, mh[:, 0:512], xv)
                else:
                    nc.vector.tensor_mul(dst, src_psum, xv)

            def a_phase(first):
                """logits (+)= a;  a_c[r,b] = sum_i x*(W_c^T v_c).

                Capsule pairs share each 512-wide psum bank; chunk t=8 (the
                16-row logit tail) runs first so its softmax half can overlap
                the rest of the phase.
                """
                for cp in range(5):
                    c0 = 2 * cp
                    pa1 = ps_a.tile([128, 512], F32, tag="pa")
                    pa2 = ps_a2.tile([16, 512], F32, tag="pa2")
                    for t in (8, 0, 1, 2, 3, 4, 5, 6, 7):
                        mp = ps_big.tile([128, 512], F32, tag="big")
                        if cp < 4:
                            nc.tensor.matmul(
                                mp[:, 0:512],
                                WZ[:, cp, 128 * t:128 * (t + 1)],
                                VCP[:, :], start=True, stop=True)
                        else:
                            nc.tensor.matmul(
                                mp[:, 0:512],
                                WZ2[:, 128 * t:128 * (t + 1)],
                                VCP2[:, :], start=True, stop=True)
                        xv = _insert_bcast(XT[:, t, :], 1, 2)
                        pr = prp.tile([128, 512], F16, tag="prod")
                        fold(pr[:, 0:512], mp[:, 0:512], xv, cp < 3)
                        if t < 8:
                            nc.tensor.matmul(pa1[:, :], SEL[:, 1, t, :],
                                             pr[:, 0:512],
                                             start=(t == 0), stop=(t == 7))
                        else:
                            nc.tensor.matmul(pa2[:, :], SEL[:, 1, 8, 0:16],
                                             pr[:, 0:512],
                                             start=True, stop=True)
                            # tail logits done early -> softmax half overlaps
                            blv = BL2[:, c0:c0 + 2, :].rearrange(
                                "p c b -> p (c b)")
                            if first:
                                nc.scalar.copy(blv, pa2[:, :])
                            else:
                                nc.vector.tensor_add(blv, blv, pa2[:, :])
                    blv = BL1[:, c0:c0 + 2, :].rearrange("p c b -> p (c b)")
                    if first:
                        nc.scalar.copy(blv, pa1[:, :])
                    else:
                        nc.vector.tensor_add(blv, blv, pa1[:, :])

            def softmax(first):
                """en = softmax_c(logits) -> fp16, transposed layout.

                first=True: logits are bounded (|a1|<~90, row-max>-12), so
                skip the max pass and use exp(b-40) in f32.
                """
                def red(dst, src3, op):
                    v = src3.rearrange("p c b -> p b c")
                    nc.vector.tensor_reduce(dst[:, :], v[:, :, :],
                                            axis=AX, op=op)

                def zsum(dst, src3):
                    # sum over c on GpSimd (frees the DVE): tree of adds
                    nc.gpsimd.tensor_add(dst[:, :], src3[:, 0, :],
                                         src3[:, 1, :])
                    for c in range(2, C):
                        nc.gpsimd.tensor_add(dst[:, :], dst[:, :],
                                             src3[:, c, :])

                if first:
                    nc.scalar.activation(EF2[:, :, :], BL2[:, :, :], Exp,
                                         bias=NEG40[0:16, 0:1])
                    red(Z2, EF2[:, :, :], ADD)
                    nc.vector.reciprocal_approx_fast(Z2[:, :], Z2[:, :])
                    nc.vector.tensor_mul(EN2[:, :, :], EF2[:, :, :],
                                         _insert_bcast(Z2[:, :], 1, C))
                    nc.scalar.activation(EF1[:, :, :], BL1[:, :, :], Exp,
                                         bias=NEG40[:, 0:1])
                    red(Z1, EF1[:, :, :], ADD)
                    nc.vector.reciprocal_approx_fast(Z1[:, :], Z1[:, :])
                    nc.vector.tensor_mul(EN1[:, :, :], EF1[:, :, :],
                                         _insert_bcast(Z1[:, :], 1, C))
                    return
                red(MX2, BL2[:, :, :], MAX)
                nc.vector.tensor_sub(E2[:, :, :], BL2[:, :, :],
                                     _insert_bcast(MX2[:, :], 1, C))
                nc.scalar.activation(E2[:, :, :], E2[:, :, :], Exp)
                red(Z2, E2[:, :, :], ADD)
                nc.vector.reciprocal_approx_fast(Z2[:, :], Z2[:, :])
                nc.scalar.copy(ZH2[:, :], Z2[:, :])
                nc.vector.tensor_mul(EN2[:, :, :], E2[:, :, :],
                                     _insert_bcast(ZH2[:, :], 1, C))
                red(MX1, BL1[:, :, :], MAX)
                nc.vector.tensor_sub(E1[:, :, :], BL1[:, :, :],
                                     _insert_bcast(MX1[:, :], 1, C))
                nc.scalar.activation(E1[:, :, :], E1[:, :, :], Exp)
                red(Z1, E1[:, :, :], ADD)
                nc.vector.reciprocal_approx_fast(Z1[:, :], Z1[:, :])
                nc.scalar.copy(ZH1[:, :], Z1[:, :])
                nc.vector.tensor_mul(EN1[:, :, :], E1[:, :, :],
                                     _insert_bcast(ZH1[:, :], 1, C))

            def s_phase():
                """y = repl(en)*x; paired block s-matmul [32,512] per (cp,t);
                only the diagonal 16x256 blocks of each pair are kept."""
                writes = []
                for cp in range(5):
                    c0 = 2 * cp
                    spp = ps_sp.tile([32, 512], F32, tag="sp")
                    for t in range(NT):
                        rp = ps_big.tile([128, 512], F32, tag="big")
                        if t < 8:
                            nc.tensor.matmul(
                                rp[:, 0:512], SEL[:, 0, t, :],
                                EN1[:, c0:c0 + 2, :].rearrange(
                                    "p c b -> p (c b)"),
                                start=True, stop=True)
                        else:
                            nc.tensor.matmul(
                                rp[:, 0:512], SEL[0:16, 0, 8, :],
                                EN2[:, c0:c0 + 2, :].rearrange(
                                    "p c b -> p (c b)"),
                                start=True, stop=True)
                        xv = _insert_bcast(XT[:, t, :], 1, 2)
                        y = yp.tile([128, 512], F16, tag="y")
                        fold(y[:, 0:512], rp[:, 0:512], xv, cp < 3)
                        nc.tensor.matmul(
                            spp[:, :], WS[:, t, 32 * cp:32 * (cp + 1)],
                            y[:, 0:512], start=(t == 0), stop=(t == NT - 1))
                    writes.append((spp[0:16, 0:256], 16, 32 * cp))
                    writes.append((spp[0:32, 256:512], 32, 32 * cp + 16))
                return writes

            def pe_warm(n):
                """keep the PE clock-gate warm across engine-idle windows"""
                jt = ps_big.tile([128, 512], F32, tag="big")
                for _ in range(n):
                    nc.tensor.matmul(jt[:, 0:256], SEL[:, 0, 0, :],
                                     XT[:, 0, 0:256], start=True, stop=True)

            # ---------------- routing ----------------
            if collectives:
                # tiny collective issued first: absorbs the one-time CC
                # barrier + stream-start latency off the critical path
                d_in = dram.tile([8, 8], F32, tag="warm_in")
                d_out = dram.tile([8, 8], F32, tag="warm_out")
                wtile = work.tile([8, 8], F32, tag="warm")
                nc.vector.memset(wtile, 0.0)
                nc.scalar.dma_start(out=d_in[:], in_=wtile[:, :])
                nc.gpsimd.collective_compute(
                    "AllReduce", ADD,
                    replica_groups=[list(range(NCORES))],
                    ins=[d_in[:].opt()], outs=[d_out[:].opt()])
            s0_local()
            for it in range(1, n_iters):
                a_phase(first=(it == 1))
                softmax(first=(it == 1))
                writes = s_phase()
                last = (it == n_iters - 1)
                bout = reduce_s(writes, last=last)
                if last:
                    squash_shard(bout)
                else:
                    squash_full(bout)

    nc.compile()
    return nc


def prep_inputs(x: np.ndarray, W: np.ndarray):
    """Host-side layout prep. Returns per-core input dicts."""
    W = W[0]  # [R, C, O, I]
    f16 = np.float16
    # selector matrices, shared by all cores
    p = np.arange(128)[:, None]
    m = np.arange(128)[None, :]
    sel = np.zeros((128, 2, NT, 128), np.float32)
    for t in range(NT):
        sel[:, 0, t, :] = (p == (16 * t + m // 8) % 128) & \
            ((16 * t + m // 8) // 128 == t // 8)
        sel[:, 1, t, :] = (m == (16 * t + p // 8) % 128) & \
            ((16 * t + p // 8) // 128 == t // 8)
    selh = np.ascontiguousarray(sel.astype(f16))
    # full (replicated) x / W for the local iteration-0 pass
    xtf = np.transpose(x, (1, 2, 0)).reshape(NTF, 128, B)
    xtf = np.ascontiguousarray(np.transpose(xtf, (1, 0, 2))).astype(f16)
    wsf = np.transpose(W.reshape(NTF, 16, C, O, I), (0, 1, 4, 2, 3))
    wsf = wsf.reshape(NTF, 128, CO)
    wsf = np.ascontiguousarray(np.transpose(wsf, (1, 0, 2))).astype(f16)
    in_maps = []
    for k in range(NCORES):
        rs = slice(k * RL, (k + 1) * RL)
        xk = np.ascontiguousarray(x[:, rs, :])      # [B, RL, I]
        wk = np.ascontiguousarray(W[rs])            # [RL, C, O, I]
        xt = np.transpose(xk, (1, 2, 0)).reshape(NT, 128, B)
        xt = np.transpose(xt, (1, 0, 2))            # [128, NT, B]
        xt = np.concatenate([xt, xt], axis=2)       # [128, NT, 2B] dup
        # ws[p, t, c*16+o] = W[16t + p//8, c, o, p%8]
        wsk = np.transpose(wk.reshape(NT, 16, C, O, I), (0, 1, 4, 2, 3))
        wsk = wsk.reshape(NT, 128, CO)
        wsk = np.transpose(wsk, (1, 0, 2))          # [128, NT, CO]
        # wot[o, c, r*8+i] = W[r, c, o, i]; zero-pad into (c8, o) rows
        wot = np.transpose(wk, (2, 1, 0, 3)).reshape(O, C, RI)
        wz = np.zeros((128, 5, RI), np.float32)
        for c in range(8):
            wz[16 * c:16 * (c + 1), c // 2, :] = wot[:, c, :]
        wz2 = np.zeros((32, RI), np.float32)
        for c in (8, 9):
            wz2[16 * (c - 8):16 * (c - 7), :] = wot[:, c, :]
        pm = np.arange(128)[:, None] % 32 < 16
        nm = np.arange(512)[None, :] < 256
        mk = (pm == nm).astype(f16)
        in_maps.append({
            "xtf": np.ascontiguousarray(np.roll(xtf, -NT * k, axis=1)),
            "wsf": np.ascontiguousarray(np.roll(wsf, -NT * k, axis=1)),
            "wz": np.ascontiguousarray(wz).astype(f16),
            "wz2": np.ascontiguousarray(wz2).astype(f16),
            "sel": selh,
            "mk": mk,
        })
    return in_maps


_CACHE = {}


def _get_nc(n_iters: int):
    if n_iters not in _CACHE:
        _CACHE[n_iters] = build_kernel(n_iters)
    return _CACHE[n_iters]


def kernel(x, W, num_iterations, _trace=False):
    n = int(num_iterations)
    assert n >= 2, "n_iters==1 not built (problem uses 3)"
    nc = _get_nc(n)
    in_maps = prep_inputs(np.asarray(x, dtype=np.float32),
                          np.asarray(W, dtype=np.float32))
    res = run_bass_kernel_spmd(nc, in_maps, list(range(NCORES)),
                               trace=_trace)
    full = np.concatenate([res.results[k]["out"] for k in range(NCORES)],
                          axis=0)                       # [160, B] co-major
    v = np.transpose(full.reshape(C, O, B), (2, 0, 1))[..., None]
    kernel.last_results = res
    return v.astype(np.float32)
